# revision 1
# baseline (speedup 1.0000x reference)
"""Self-contained TRN2 Bass kernel for nn_AESModel_42760694399363.

2-layer NF4-quantized transformer (B=4,S=1024,D=2048,FF=8192,H=16) + mean-pool
+ linear head.  Tensor-parallel across 8 NeuronCores (Megatron-SP style):
q/k/v/gate/up column-sharded, o/down row-sharded, heads data-parallel in
attention, residual stream sequence-sharded with AllGather / ReduceScatter.
Host does the embedding gather and NF4 dequant (pure data prep); all matmul /
attention / norm FLOPs run on device in bf16 with f32 accumulation.
"""
import sys

sys.path.insert(0, "/opt/trn_rl_repo")

import numpy as np
import ml_dtypes

# ---------------------------------------------------------------- constants
B, S, D, L, FF, V, NS = 4, 1024, 2048, 2, 8192, 32000, 11
H, DH = 16, 128
NCORES = 8
TOK = B * S              # 4096
SHARD = TOK // NCORES    # 512
DQ = D // NCORES         # 256  q/k/v out-cols per core
FFC = FF // NCORES       # 1024 gate/up cols per core
KC = D // 128            # 16 contraction chunks over D
SCALE = 1.0 / np.sqrt(DH)
EPS = 1e-5
ROPE_THETA = 10000.0
BLK = 64

NF4 = np.array([-1.0, -0.6961928009986877, -0.5250730514526367, -0.39491748809814453,
                -0.28444138169288635, -0.18477343022823334, -0.09105003625154495, 0.0,
                0.07958029955625534, 0.16093020141124725, 0.24611230194568634,
                0.33791524171829224, 0.44070982933044434, 0.5626170039176941,
                0.7229568362236023, 1.0], dtype=np.float32)

BF = ml_dtypes.bfloat16

_CACHE = {}


# ---------------------------------------------------------------- device graph
def build_graph(stage=4):
    import concourse.mybir as mybir
    import concourse.tile as tile
    from concourse import bacc
    from concourse.masks import make_identity

    F32 = mybir.dt.float32
    BF16 = mybir.dt.bfloat16
    RG = [list(range(NCORES))]
    Exp = mybir.ActivationFunctionType.Exp
    Sigmoid = mybir.ActivationFunctionType.Sigmoid

    from concourse.tile_rust import add_dep_helper

    nc = bacc.Bacc("TRN2", target_bir_lowering=False, debug=False,
                   num_devices=NCORES)
    _prev_cc = [None]

    def _chain_cc(cc):
        # never allow two collectives in flight: serialize in issue order
        if _prev_cc[0] is not None:
            add_dep_helper(cc.ins, _prev_cc[0], reason="serialize collectives")
        _prev_cc[0] = cc.ins

    x_ext = nc.declare_dram_parameter("x", [SHARD, D], F32, isOutput=False)
    wq_ext = nc.declare_dram_parameter("wq", [L, D, DQ], BF16, isOutput=False)
    wk_ext = nc.declare_dram_parameter("wk", [L, D, DQ], BF16, isOutput=False)
    wv_ext = nc.declare_dram_parameter("wv", [L, D, DQ], BF16, isOutput=False)
    wo_ext = nc.declare_dram_parameter("wo", [L, DQ, D], BF16, isOutput=False)
    wg_ext = nc.declare_dram_parameter("wg", [L, D, FFC], BF16, isOutput=False)
    wu_ext = nc.declare_dram_parameter("wu", [L, D, FFC], BF16, isOutput=False)
    wd_ext = nc.declare_dram_parameter("wd", [L, FFC, D], BF16, isOutput=False)
    cosf_ext = nc.declare_dram_parameter("cosf", [128, TOK], BF16, isOutput=False)
    sinf_ext = nc.declare_dram_parameter("sinf", [128, TOK], BF16, isOutput=False)
    cm_ext = nc.declare_dram_parameter("cmask", [4, 128, 512], BF16, isOutput=False)
    hw_ext = nc.declare_dram_parameter("hw", [D, NS], F32, isOutput=False)
    out_ext = nc.declare_dram_parameter("out", [NS, 1], F32, isOutput=True)

    with tile.TileContext(nc) as tc:
        with tc.tile_pool(name="const", bufs=1) as constp, \
             tc.tile_pool(name="xres", bufs=1) as xres, \
             tc.tile_pool(name="norm", bufs=1) as normp, \
             tc.tile_pool(name="small", bufs=4) as small, \
             tc.tile_pool(name="drain", bufs=4) as drain, \
             tc.tile_pool(name="psmm", bufs=4, space="PSUM") as psmm, \
             tc.tile_pool(name="pstr", bufs=2, space="PSUM") as pstr, \
             tc.tile_pool(name="psq", bufs=1, space="PSUM") as psq, \
             tc.tile_pool(name="dram", bufs=2, space="DRAM") as dram:

            ident = constp.tile([128, 128], BF16, tag="ident")
            make_identity(nc, ident[:])
            cmask = [constp.tile([128, 512], BF16, tag=f"cmask{d}",
                                 name=f"cmask{d}") for d in range(4)]
            for d in range(4):
                nc.sync.dma_start(cmask[d][:], cm_ext[d])

            # residual stream, f32, [128 part, 4 s-tiles, D]
            x_sb = xres.tile([128, 4, D], F32, tag="x")
            nc.sync.dma_start(
                x_sb[:], x_ext.ap().rearrange("(t p) d -> p t d", p=128))

            def rmsnorm_bf16(t, h_out):
                """h_out[128, D] bf16 = x_sb[:, t, :] * rsqrt(mean(x^2)+eps)."""
                ssq = small.tile([128, 1], F32, tag="ssq")
                nc.scalar.activation(h_out[:], x_sb[:, t, :],
                                     mybir.ActivationFunctionType.Square,
                                     accum_out=ssq[:])
                ms = small.tile([128, 1], F32, tag="ms")
                nc.vector.tensor_scalar(ms[:], ssq[:], 1.0 / D, EPS,
                                        mybir.AluOpType.mult,
                                        mybir.AluOpType.add)
                st = small.tile([128, 1], F32, tag="st")
                nc.scalar.sqrt(st[:], ms[:])
                rstd = small.tile([128, 1], F32, tag="rstd")
                nc.vector.reciprocal(rstd[:], st[:])
                nc.vector.tensor_scalar_mul(h_out[:], x_sb[:, t, :], rstd[:])

            def norm_transpose_chunk(t, hct):
                """rmsnorm s-tile t -> 16 PE transposes -> hct cols block t."""
                hbf = normp.tile([128, D], BF16, tag="hbf")
                rmsnorm_bf16(t, hbf)
                for kq in range(KC // 4):
                    pt = pstr.tile([128, 4, 128], BF16, tag="tr")
                    for q in range(4):
                        nc.tensor.transpose(
                            pt[:, q, :],
                            hbf[:, 512 * kq + 128 * q:512 * kq + 128 * (q + 1)],
                            ident[:])
                    hts = drain.tile([128, 4, 128], BF16, tag="ob")
                    nc.vector.tensor_copy(hts[:], pt[:])
                    nc.sync.dma_start(
                        hct[512 * kq:512 * (kq + 1),
                            128 * t:128 * (t + 1)].rearrange(
                                "(q p) n -> p q n", p=128), hts[:])

            def allgather(hct, tagsuf):
                hall = dram.tile([NCORES * D, SHARD], BF16, tag="hall",
                                 name="hall" + tagsuf, addr_space="Shared")
                cc = nc.gpsimd.collective_compute(
                    "AllGather", mybir.AluOpType.bypass, replica_groups=RG,
                    ins=[hct.opt()], outs=[hall.opt()])
                _chain_cc(cc)
                return hall

            def residual_chunk(t, rs_out_k):
                """x_sb[:, t, :] += rs_out_k [128, D] (bf16 in DRAM)."""
                for n in range(4):
                    db = drain.tile([128, 512], BF16, tag="ob")
                    nc.sync.dma_start(
                        db[:], rs_out_k[:, 512 * n:512 * (n + 1)])
                    nc.vector.tensor_add(
                        x_sb[:, t, 512 * n:512 * (n + 1)],
                        x_sb[:, t, 512 * n:512 * (n + 1)], db[:])

            def make_rs_bufs(tagsuf):
                rs_in = dram.tile([TOK, D], BF16, tag="rsin",
                                  name="rsin_" + tagsuf)
                rs_out = dram.tile([SHARD, D], BF16, tag="rsout",
                                   name="rsout_" + tagsuf)
                return rs_in, rs_out

            def rs_all(rs_in, rs_out):
                cc = nc.gpsimd.collective_compute(
                    "ReduceScatter", mybir.AluOpType.add, replica_groups=RG,
                    ins=[rs_in.opt()], outs=[rs_out.opt()])
                _chain_cc(cc)

            def post_rs(rs_out, hct):
                """Per s-tile: residual add, then (optionally) norm+transpose."""
                for k in range(4):
                    residual_chunk(k, rs_out[128 * k:128 * (k + 1), :])
                    if hct is not None:
                        norm_transpose_chunk(k, hct)

            # layer-0 attention input
            hct0 = dram.tile([D, SHARD], BF16, tag="hct", name="hct_l0")
            for t in range(4):
                norm_transpose_chunk(t, hct0)
            hall = allgather(hct0, "a0")

            for l in range(L):
                # ======================= attention =======================
                if 2 * l + 1 > stage:
                    break
                with tc.tile_pool(name=f"attn{l}", bufs=1) as ap, \
                     tc.tile_pool(name=f"attn2{l}", bufs=1) as ap2, \
                     tc.tile_pool(name=f"attnpT{l}", bufs=1) as appt:
                    cosf = ap.tile([128, TOK], BF16, tag="cosf")
                    nc.sync.dma_start(cosf[:], cosf_ext[:])
                    sinf = ap.tile([128, TOK], BF16, tag="sinf")
                    nc.sync.dma_start(sinf[:], sinf_ext[:])
                    wq_sb = ap.tile([128, KC, DQ], BF16, tag="wq")
                    nc.sync.dma_start(
                        wq_sb[:], wq_ext[l].rearrange("(c p) m -> p c m", p=128))
                    wk_sb = ap.tile([128, KC, DQ], BF16, tag="wk")
                    nc.sync.dma_start(
                        wk_sb[:], wk_ext[l].rearrange("(c p) m -> p c m", p=128))
                    wv_sb = ap.tile([128, KC, DQ], BF16, tag="wv")
                    nc.sync.dma_start(
                        wv_sb[:], wv_ext[l].rearrange("(c p) m -> p c m", p=128))
                    wo_sb = ap.tile([128, 2, D], BF16, tag="wo")
                    nc.sync.dma_start(
                        wo_sb[:], wo_ext[l].rearrange("(c p) m -> p c m", p=128))


                    rs_in, rs_out = make_rs_bufs(f"a{l}")
                    oT = ap.tile([128, 2, TOK], BF16, tag="oT")
                    for b in range(B):
                        # ---- qkv for this batch's two 512-token blocks
                        qTb = ap.tile([128, 2, S], BF16, tag="qTb")
                        kTb = ap.tile([128, 2, S], BF16, tag="kTb")
                        v_b = ap.tile([128, 8, DQ], BF16, tag="vb")
                        for rr in range(2):
                            r = 2 * b + rr
                            h_sb = ap.tile([128, KC, SHARD], BF16, tag="hr")
                            nc.sync.dma_start(
                                h_sb[:],
                                hall[D * r:D * (r + 1), :].rearrange(
                                    "(c p) n -> p c n", p=128))
                            cs = cosf[:, SHARD * r:SHARD * (r + 1)]
                            sn = sinf[:, SHARD * r:SHARD * (r + 1)]
                            for wsb, dstT in ((wq_sb, qTb), (wk_sb, kTb)):
                                for m in range(2):
                                    pq = psmm.tile([128, SHARD], F32, tag="mm")
                                    for kc in range(KC):
                                        nc.tensor.matmul(
                                            pq[:],
                                            wsb[:, kc, 128 * m:128 * (m + 1)],
                                            h_sb[:, kc, :],
                                            start=(kc == 0), stop=(kc == KC - 1))
                                    ta = ap2.tile([128, SHARD], F32, tag="ropea")
                                    nc.vector.tensor_mul(ta[:], pq[:], cs)
                                    tb = ap2.tile([128, SHARD], F32, tag="ropeb")
                                    nc.vector.tensor_mul(
                                        tb[0:64, :], pq[64:128, :], sn[0:64, :])
                                    nc.vector.tensor_mul(
                                        tb[64:128, :], pq[0:64, :],
                                        sn[64:128, :])
                                    nc.vector.tensor_add(
                                        dstT[:, m,
                                             SHARD * rr:SHARD * (rr + 1)],
                                        ta[:], tb[:])
                            for t in range(4):
                                pv = psmm.tile([128, DQ], F32, tag="mm")
                                for kc in range(KC):
                                    nc.tensor.matmul(
                                        pv[:],
                                        h_sb[:, kc, 128 * t:128 * (t + 1)],
                                        wv_sb[:, kc, :],
                                        start=(kc == 0), stop=(kc == KC - 1))
                                nc.vector.tensor_copy(
                                    v_b[:, 4 * rr + t, :], pv[:])
                        # ---- attention for b, per local head
                        for hh in range(2):
                            qh = qTb[:, hh, :]
                            kh = kTb[:, hh, :]
                            pT_sb = appt.tile([128, 8, S], BF16, tag="pT")
                            for i in range(8):
                                nv = 512 if i < 4 else 1024
                                sc = ap2.tile([128, S], F32, tag="sc", bufs=2)
                                for j in range(nv // 512):
                                    ps_s = psmm.tile([128, 512], F32, tag="mm")
                                    nc.tensor.matmul(
                                        ps_s[:], qh[:, 128 * i:128 * (i + 1)],
                                        kh[:, 512 * j:512 * (j + 1)],
                                        start=True, stop=True)
                                    if j == i // 4:
                                        nc.vector.tensor_add(
                                            sc[:, 512 * j:512 * (j + 1)],
                                            ps_s[:], cmask[i % 4][:])
                                    else:
                                        nc.vector.tensor_copy(
                                            sc[:, 512 * j:512 * (j + 1)],
                                            ps_s[:])
                                mx = small.tile([128, 1], F32, tag="mx")
                                nc.vector.reduce_max(
                                    mx[:], sc[:, :nv],
                                    axis=mybir.AxisListType.X)
                                nmx = small.tile([128, 1], F32, tag="nmx")
                                nc.vector.tensor_scalar_mul(
                                    nmx[:], mx[:], -SCALE)
                                pb = ap2.tile([128, S], BF16, tag="pb", bufs=2)
                                se = small.tile([128, 1], F32, tag="se")
                                nc.scalar.activation(
                                    pb[:, :nv], sc[:, :nv], Exp,
                                    bias=nmx[:], scale=SCALE, accum_out=se[:])
                                rse = small.tile([128, 1], F32, tag="rse")
                                nc.vector.reciprocal(rse[:], se[:])
                                nc.vector.tensor_scalar_mul(
                                    pb[:, :nv], pb[:, :nv], rse[:])
                                for tq in range(nv // 512):
                                    pt = pstr.tile([128, 4, 128], BF16,
                                                   tag="tr")
                                    for q in range(4):
                                        nc.tensor.transpose(
                                            pt[:, q, :],
                                            pb[:, 512 * tq + 128 * q:
                                               512 * tq + 128 * (q + 1)],
                                            ident[:])
                                    nc.vector.tensor_copy(
                                        pT_sb[:, 4 * tq:4 * (tq + 1),
                                              128 * i:128 * (i + 1)], pt[:])
                            for ns in range(2):
                                po = psmm.tile([128, 512], F32, tag="mm")
                                kmax = 4 if ns == 0 else 8
                                for tcb in range(kmax):
                                    nc.tensor.matmul(
                                        po[:],
                                        v_b[:, tcb, 128 * hh:128 * (hh + 1)],
                                        pT_sb[:, tcb, 512 * ns:512 * (ns + 1)],
                                        start=(tcb == 0), stop=(tcb == kmax - 1))
                                nc.vector.tensor_copy(
                                    oT[:, hh, S * b + 512 * ns:
                                       S * b + 512 * (ns + 1)], po[:])

                    # ---- o_proj partials over all 32 s-tiles
                    for sg in range(32):
                        for n in range(4):
                            pp = psmm.tile([128, 512], F32, tag="mm")
                            for kc in range(2):
                                nc.tensor.matmul(
                                    pp[:],
                                    oT[:, kc, 128 * sg:128 * (sg + 1)],
                                    wo_sb[:, kc, 512 * n:512 * (n + 1)],
                                    start=(kc == 0), stop=(kc == 1))
                            ob = drain.tile([128, 512], BF16, tag="ob")
                            if n % 2 == 0:
                                nc.vector.tensor_copy(ob[:], pp[:])
                            else:
                                nc.scalar.copy(ob[:], pp[:])
                            nc.sync.dma_start(
                                rs_in[128 * sg:128 * (sg + 1),
                                      512 * n:512 * (n + 1)], ob[:])
                    rs_all(rs_in, rs_out)
                    # mlp input AG built chunk-by-chunk behind the RS pipeline
                    hct = dram.tile([D, SHARD], BF16, tag="hct",
                                    name=f"hct_m{l}")
                    post_rs(rs_out, hct)
                    hall2 = allgather(hct, f"m{l}")

                # ========================= MLP ==========================
                if 2 * l + 2 > stage:
                    break
                with tc.tile_pool(name=f"mlp{l}", bufs=1) as mp, \
                     tc.tile_pool(name=f"mlp2{l}", bufs=2) as mp2, \
                     tc.tile_pool(name=f"mlp3{l}", bufs=3) as mp3:
                    wg_sb = mp.tile([128, KC, FFC], BF16, tag="wg")
                    nc.sync.dma_start(
                        wg_sb[:], wg_ext[l].rearrange("(c p) m -> p c m", p=128))
                    wu_sb = mp.tile([128, KC, FFC], BF16, tag="wu")
                    nc.sync.dma_start(
                        wu_sb[:], wu_ext[l].rearrange("(c p) m -> p c m", p=128))

                    rs2_in, rs2_out = make_rs_bufs(f"m{l}")
                    for r in range(NCORES):
                        h_sb = mp2.tile([128, KC, SHARD], BF16, tag="hr")
                        nc.sync.dma_start(
                            h_sb[:],
                            hall2[D * r:D * (r + 1), :].rearrange(
                                "(c p) n -> p c n", p=128))
                        aT = mp2.tile([128, 8, SHARD], BF16, tag="aT")
                        for m in range(8):
                            pg = psmm.tile([128, SHARD], F32, tag="mm")
                            for kc in range(KC):
                                nc.tensor.matmul(
                                    pg[:], wg_sb[:, kc, 128 * m:128 * (m + 1)],
                                    h_sb[:, kc, :],
                                    start=(kc == 0), stop=(kc == KC - 1))
                            pu = psmm.tile([128, SHARD], F32, tag="mm")
                            for kc in range(KC):
                                nc.tensor.matmul(
                                    pu[:], wu_sb[:, kc, 128 * m:128 * (m + 1)],
                                    h_sb[:, kc, :],
                                    start=(kc == 0), stop=(kc == KC - 1))
                            sg_t = mp2.tile([128, SHARD], F32, tag="silu")
                            nc.scalar.activation(sg_t[:], pg[:], Sigmoid)
                            nc.vector.tensor_mul(sg_t[:], sg_t[:], pg[:])
                            nc.vector.tensor_mul(aT[:, m, :], sg_t[:], pu[:])
                        for n in range(4):
                            pds = [psmm.tile([128, 512], F32, tag="mm",
                                             name=f"pd{r}_{n}_{t}")
                                   for t in range(4)]
                            for kc in range(8):
                                wdt = mp3.tile([128, 512], BF16, tag="wdt")
                                nc.sync.dma_start(
                                    wdt[:],
                                    wd_ext[l, 128 * kc:128 * (kc + 1),
                                           512 * n:512 * (n + 1)])
                                for t in range(4):
                                    nc.tensor.matmul(
                                        pds[t][:],
                                        aT[:, kc, 128 * t:128 * (t + 1)],
                                        wdt[:],
                                        start=(kc == 0), stop=(kc == 7))
                            for t in range(4):
                                ob = drain.tile([128, 512], BF16, tag="ob")
                                nc.vector.tensor_copy(ob[:], pds[t][:])
                                nc.sync.dma_start(
                                    rs2_in[128 * (4 * r + t):
                                           128 * (4 * r + t + 1),
                                           512 * n:512 * (n + 1)], ob[:])
                    rs_all(rs2_in, rs2_out)
                    if l < L - 1:
                        hct = dram.tile([D, SHARD], BF16, tag="hct",
                                        name=f"hct_a{l + 1}")
                        post_rs(rs2_out, hct)
                        hall = allgather(hct, f"a{l + 1}")
                    else:
                        post_rs(rs2_out, None)

            # ===================== final norm + pool + head ==============
            with tc.tile_pool(name="fin", bufs=1) as finp:
                hw_sb = constp.tile([128, KC, NS], F32, tag="hw")
                nc.sync.dma_start(
                    hw_sb[:], hw_ext.ap().rearrange("(c p) m -> p c m", p=128))
                pvec = constp.tile([128, 128], BF16, tag="pvec")
                nc.vector.memset(pvec[:], 1.0 / S)
                pooled = small.tile([1, D], F32, tag="pooled")
                hfins = []
                for t in range(4):
                    hfin = finp.tile([128, D], BF16, tag=f"hfin{t}",
                                     name=f"hfin{t}")
                    rmsnorm_bf16(t, hfin)
                    hfins.append(hfin)
                for n in range(4):
                    pq = psq.tile([128, 512], F32, tag="pool", name=f"pool{n}")
                    for t in range(4):
                        nc.tensor.matmul(pq[:], pvec[:],
                                         hfins[t][:, 512 * n:512 * (n + 1)],
                                         start=(t == 0), stop=(t == 3))
                    nc.vector.tensor_copy(
                        pooled[0:1, 512 * n:512 * (n + 1)], pq[0:1, :])
                pdram = dram.tile([1, D], F32, tag="pdram")
                nc.sync.dma_start(pdram[:], pooled[:])
                pooledT = small.tile([128, KC], F32, tag="pooledT")
                nc.sync.dma_start(
                    pooledT[:],
                    pdram[0:1, :].rearrange("a (c p) -> p (a c)", p=128))
                py = psq.tile([NS, 1], F32, tag="head")
                for kc in range(KC):
                    nc.tensor.matmul(
                        py[:], hw_sb[:, kc, :], pooledT[:, kc:kc + 1],
                        start=(kc == 0), stop=(kc == KC - 1))
                y_sb = small.tile([NS, 1], F32, tag="y")
                nc.vector.tensor_copy(y_sb[:], py[:])
                nc.sync.dma_start(out_ext[:], y_sb[:])

    nc.compile()
    return nc


# ---------------------------------------------------------------- host prep
def _dequant(codes, scales):
    """codes uint8 [..., O, I], scales f32 [..., O, I//BLK] -> f32 [..., O, I]."""
    w = NF4[codes]
    shp = w.shape
    w = w.reshape(shp[:-1] + (shp[-1] // BLK, BLK)) * scales[..., None]
    return w.reshape(shp).astype(np.float32)


def prep_inputs(inputs):
    """Full inputs -> per-core in_maps."""
    ii = np.asarray(inputs["input_ids"])
    embed = np.asarray(inputs["embed"], dtype=np.float32)
    x0 = embed[ii].reshape(TOK, D)

    attn_w = _dequant(np.asarray(inputs["attn_codes"]),
                      np.asarray(inputs["attn_scales"], dtype=np.float32))
    gu_w = _dequant(np.asarray(inputs["gu_codes"]),
                    np.asarray(inputs["gu_scales"], dtype=np.float32))
    down_w = _dequant(np.asarray(inputs["down_codes"]),
                      np.asarray(inputs["down_scales"], dtype=np.float32))
    anw = np.asarray(inputs["attn_norm_w"], dtype=np.float32)   # [L, D]
    mnw = np.asarray(inputs["mlp_norm_w"], dtype=np.float32)    # [L, D]
    fnw = np.asarray(inputs["final_norm_w"], dtype=np.float32)  # [D]
    head_w = np.asarray(inputs["head_w"], dtype=np.float32)     # [NS, D]

    # rope basis permutation within each head: [even dims, odd dims]
    perm = np.concatenate([np.arange(0, DH, 2), np.arange(1, DH, 2)])

    # rope tables, extended over the gathered column order (r, s_local)
    pos = np.concatenate(
        [512 * (r % 2) + np.arange(SHARD) for r in range(NCORES)])  # [TOK]
    inv = 1.0 / (ROPE_THETA ** (np.arange(0, DH, 2, dtype=np.float32) / DH))
    ang = inv[:, None] * pos[None, :].astype(np.float32)            # [64, TOK]
    cosf = np.concatenate([np.cos(ang), np.cos(ang)], axis=0).astype(BF)
    sinf = np.concatenate([-np.sin(ang), np.sin(ang)], axis=0).astype(BF)

    # causal mask tiles: cmask[d, sl, tl] = 0 if 128*d + sl >= tl else -1e9
    sl = np.arange(128)[:, None]
    tl = np.arange(512)[None, :]
    cmask = np.stack([np.where(128 * d + sl >= tl, 0.0, -1e9)
                      for d in range(4)]).astype(BF)

    hwp = (head_w * fnw[None, :]).T.copy().astype(np.float32)       # [D, NS]

    in_maps = []
    for c in range(NCORES):
        m = {}
        m["x"] = np.ascontiguousarray(x0[SHARD * c:SHARD * (c + 1)])
        rows = slice(DQ * c, DQ * (c + 1))
        prows = np.concatenate([perm + DH * h for h in (0, 1)]) + DQ * c
        # fold input-side rmsnorm weight into the projection weights
        wq = np.stack([(attn_w[l, 0][prows] * anw[l][None, :]).T for l in range(L)])
        wk = np.stack([(attn_w[l, 1][prows] * anw[l][None, :]).T for l in range(L)])
        wv = np.stack([(attn_w[l, 2][rows] * anw[l][None, :]).T for l in range(L)])
        wo = np.stack([attn_w[l, 3][:, rows].T for l in range(L)])
        frows = slice(FFC * c, FFC * (c + 1))
        wg = np.stack([(gu_w[l, 0][frows] * mnw[l][None, :]).T for l in range(L)])
        wu = np.stack([(gu_w[l, 1][frows] * mnw[l][None, :]).T for l in range(L)])
        wd = np.stack([down_w[l][:, frows].T for l in range(L)])
        for k, wmat in (("wq", wq), ("wk", wk), ("wv", wv), ("wo", wo),
                        ("wg", wg), ("wu", wu), ("wd", wd)):
            m[k] = np.ascontiguousarray(wmat).astype(BF)
        m["cosf"] = cosf
        m["sinf"] = sinf
        m["cmask"] = cmask
        m["hw"] = hwp
        in_maps.append(m)
    return in_maps


def kernel(**inputs):
    from concourse.bass_utils import run_bass_kernel_spmd

    if "nc" not in _CACHE:
        _CACHE["nc"] = build_graph()
    nc = _CACHE["nc"]
    in_maps = prep_inputs(inputs)
    res = run_bass_kernel_spmd(nc, in_maps, core_ids=list(range(NCORES)))
    ys = [res.results[c]["out"][:, 0] for c in range(NCORES)]
    head_b = np.asarray(inputs["head_b"], dtype=np.float32)
    out = np.stack([ys[2 * b] + ys[2 * b + 1] for b in range(B)]) + head_b[None, :]
    return out.astype(np.float32)



# revision 10
# speedup vs baseline: 1.2682x; 1.2682x over previous
"""Self-contained TRN2 Bass kernel for nn_AESModel_42760694399363.

2-layer NF4-quantized transformer (B=4,S=1024,D=2048,FF=8192,H=16) + mean-pool
+ linear head.  Tensor-parallel across 8 NeuronCores: q/k/v/gate/up
column-sharded, o/down row-sharded.  The residual stream is sequence-sharded
STRIDED BY BATCH: core c owns tokens {1024*q + 128*c .. +128} for each batch
q.  This makes every boundary collective a per-batch 4MB ReduceScatter /
AllGather that pipelines behind the next batch's attention (or next batch's
MLP chunk), hiding nearly all collective latency.  Host does embedding gather
and NF4 dequant; all matmul/attention/norm FLOPs run on device in bf16 with
f32 accumulation.
"""
import sys

sys.path.insert(0, "/opt/trn_rl_repo")

import numpy as np
import ml_dtypes

# ---------------------------------------------------------------- constants
B, S, D, L, FF, V, NS = 4, 1024, 2048, 2, 8192, 32000, 11
H, DH = 16, 128
NCORES = 8
TOK = B * S              # 4096
SHARD = TOK // NCORES    # 512 tokens per core (4 tiles of 128, one per batch)
DQ = D // NCORES         # 256  q/k/v out-cols per core
FFC = FF // NCORES       # 1024 gate/up cols per core
KC = D // 128            # 16 contraction chunks over D
SCALE = 1.0 / np.sqrt(DH)
EPS = 1e-5
ROPE_THETA = 10000.0
BLK = 64

NF4 = np.array([-1.0, -0.6961928009986877, -0.5250730514526367, -0.39491748809814453,
                -0.28444138169288635, -0.18477343022823334, -0.09105003625154495, 0.0,
                0.07958029955625534, 0.16093020141124725, 0.24611230194568634,
                0.33791524171829224, 0.44070982933044434, 0.5626170039176941,
                0.7229568362236023, 1.0], dtype=np.float32)

BF = ml_dtypes.bfloat16

_CACHE = {}


# ---------------------------------------------------------------- device graph
def build_graph():
    import concourse.mybir as mybir
    import concourse.tile as tile
    from concourse import bacc
    from concourse.masks import make_identity
    from concourse.tile_rust import add_dep_helper

    F32 = mybir.dt.float32
    BF16 = mybir.dt.bfloat16
    RG = [list(range(NCORES))]
    Exp = mybir.ActivationFunctionType.Exp
    Sigmoid = mybir.ActivationFunctionType.Sigmoid

    nc = bacc.Bacc("TRN2", target_bir_lowering=False, debug=False,
                   num_devices=NCORES)
    _prev_cc = [None]

    def _chain_cc(cc):
        # never allow two collectives in flight: serialize in issue order
        if _prev_cc[0] is not None:
            add_dep_helper(cc.ins, _prev_cc[0], reason="serialize collectives")
        _prev_cc[0] = cc.ins

    x_ext = nc.declare_dram_parameter("x", [SHARD, D], F32, isOutput=False)
    wq_ext = nc.declare_dram_parameter("wq", [L, D, DQ], BF16, isOutput=False)
    wk_ext = nc.declare_dram_parameter("wk", [L, D, DQ], BF16, isOutput=False)
    wv_ext = nc.declare_dram_parameter("wv", [L, D, DQ], BF16, isOutput=False)
    wo_ext = nc.declare_dram_parameter("wo", [L, DQ, D], BF16, isOutput=False)
    wg_ext = nc.declare_dram_parameter("wg", [L, D, FFC], BF16, isOutput=False)
    wu_ext = nc.declare_dram_parameter("wu", [L, D, FFC], BF16, isOutput=False)
    wd_ext = nc.declare_dram_parameter("wd", [L, FFC, D], BF16, isOutput=False)
    cosf_ext = nc.declare_dram_parameter("cosf", [128, S], BF16, isOutput=False)
    sinf_ext = nc.declare_dram_parameter("sinf", [128, S], BF16, isOutput=False)
    cm_ext = nc.declare_dram_parameter("cmask", [4, 128, 512], BF16, isOutput=False)
    hw_ext = nc.declare_dram_parameter("hw", [D, NS], F32, isOutput=False)
    out_ext = nc.declare_dram_parameter("out", [NS, B], F32, isOutput=True)

    with tile.TileContext(nc) as tc:
        with tc.tile_pool(name="const", bufs=1) as constp, \
             tc.tile_pool(name="xres", bufs=1) as xres, \
             tc.tile_pool(name="norm", bufs=1) as normp, \
             tc.tile_pool(name="small", bufs=4) as small, \
             tc.tile_pool(name="drain", bufs=4) as drain, \
             tc.tile_pool(name="psmm", bufs=4, space="PSUM") as psmm, \
             tc.tile_pool(name="pstr", bufs=2, space="PSUM") as pstr, \
             tc.tile_pool(name="psq", bufs=1, space="PSUM") as psq, \
             tc.tile_pool(name="dram", bufs=1, space="DRAM") as dram:

            ident = constp.tile([128, 128], BF16, tag="ident")
            make_identity(nc, ident[:])

            # residual stream, f32, [128 part, q=batch tile, D]
            x_sb = xres.tile([128, 4, D], F32, tag="x")
            nc.sync.dma_start(
                x_sb[:], x_ext.ap().rearrange("(t p) d -> p t d", p=128))

            def rmsnorm_bf16(t, h_out):
                """h_out[128, D] bf16 = x_sb[:, t, :] * rsqrt(mean(x^2)+eps)."""
                ssq = small.tile([128, 1], F32, tag="ssq")
                nc.scalar.activation(h_out[:], x_sb[:, t, :],
                                     mybir.ActivationFunctionType.Square,
                                     accum_out=ssq[:])
                ms = small.tile([128, 1], F32, tag="ms")
                nc.vector.tensor_scalar(ms[:], ssq[:], 1.0 / D, EPS,
                                        mybir.AluOpType.mult,
                                        mybir.AluOpType.add)
                st = small.tile([128, 1], F32, tag="st")
                nc.scalar.sqrt(st[:], ms[:])
                rstd = small.tile([128, 1], F32, tag="rstd")
                nc.vector.reciprocal(rstd[:], st[:])
                nc.vector.tensor_scalar_mul(h_out[:], x_sb[:, t, :], rstd[:])

            def norm_transpose(t, hct_q):
                """rmsnorm batch-tile t -> 16 PE transposes -> hct_q [D, 128]."""
                hbf = normp.tile([128, D], BF16, tag="hbf")
                rmsnorm_bf16(t, hbf)
                for kq in range(KC // 4):
                    pt = pstr.tile([128, 4, 128], BF16, tag="tr")
                    for q in range(4):
                        nc.tensor.transpose(
                            pt[:, q, :],
                            hbf[:, 512 * kq + 128 * q:512 * kq + 128 * (q + 1)],
                            ident[:])
                    hts = drain.tile([128, 4, 128], BF16, tag="ob")
                    nc.vector.tensor_copy(hts[:], pt[:])
                    nc.sync.dma_start(
                        hct_q[512 * kq:512 * (kq + 1), :].rearrange(
                            "(q p) n -> p q n", p=128), hts[:])

            def allgather_q(hct_q, tagsuf):
                """AG [D,128] -> [NCORES*D, 128] (4MB)."""
                hall_q = dram.tile([NCORES * D, 128], BF16, tag="hall" + tagsuf,
                                   name="hall" + tagsuf, addr_space="Shared")
                cc = nc.gpsimd.collective_compute(
                    "AllGather", mybir.AluOpType.bypass, replica_groups=RG,
                    ins=[hct_q.opt()], outs=[hall_q.opt()])
                _chain_cc(cc)
                return hall_q

            def reduce_scatter_q(rs_in_q, tagsuf):
                """RS [8*128, D] -> [128, D] (in 4MB)."""
                rs_out_q = dram.tile([128, D], BF16, tag="rso" + tagsuf,
                                     name="rso" + tagsuf)
                cc = nc.gpsimd.collective_compute(
                    "ReduceScatter", mybir.AluOpType.add, replica_groups=RG,
                    ins=[rs_in_q.opt()], outs=[rs_out_q.opt()])
                _chain_cc(cc)
                return rs_out_q

            def residual_add(t, rs_out_q):
                """x_sb[:, t, :] += rs_out_q [128, D] (bf16 in DRAM)."""
                for n in range(4):
                    db = drain.tile([128, 512], BF16, tag="ob")
                    nc.sync.dma_start(
                        db[:], rs_out_q[:, 512 * n:512 * (n + 1)])
                    nc.vector.tensor_add(
                        x_sb[:, t, 512 * n:512 * (n + 1)],
                        x_sb[:, t, 512 * n:512 * (n + 1)], db[:])

            def load_h_sb(pool, hall_q, ncol):
                """Gathered h for one batch: [128, KC, ncol*128] from hall_q."""
                h_sb = pool.tile([128, KC, ncol * 128], BF16, tag="hr")
                for r in range(ncol):
                    nc.sync.dma_start(
                        h_sb[:, :, 128 * r:128 * (r + 1)],
                        hall_q[D * r:D * (r + 1), :].rearrange(
                            "(c p) n -> p c n", p=128))
                return h_sb

            # ---------------- initial: norm+transpose+AG each batch tile
            hall = []
            for t in range(4):
                hct_q = dram.tile([D, 128], BF16, tag=f"hct_l0_{t}",
                                  name=f"hct_l0_{t}")
                norm_transpose(t, hct_q)
                hall.append(allgather_q(hct_q, f"a0_{t}"))

            for l in range(L):
                # ======================= attention =======================
                with tc.tile_pool(name=f"attn{l}", bufs=1) as ap, \
                     tc.tile_pool(name=f"attnh{l}", bufs=1) as hp, \
                     tc.tile_pool(name=f"attn2{l}", bufs=1) as ap2, \
                     tc.tile_pool(name=f"attnpT{l}", bufs=1) as appt:
                    cmask = [ap.tile([128, 512], BF16, tag=f"cmask{d}",
                                     name=f"cmask{l}_{d}") for d in range(4)]
                    for d in range(4):
                        nc.sync.dma_start(cmask[d][:], cm_ext[d])
                    cosf = ap.tile([128, S], BF16, tag="cosf")
                    nc.sync.dma_start(cosf[:], cosf_ext[:])
                    sinf = ap.tile([128, S], BF16, tag="sinf")
                    nc.sync.dma_start(sinf[:], sinf_ext[:])
                    wq_sb = ap.tile([128, KC, DQ], BF16, tag="wq")
                    nc.sync.dma_start(
                        wq_sb[:], wq_ext[l].rearrange("(c p) m -> p c m", p=128))
                    wk_sb = ap.tile([128, KC, DQ], BF16, tag="wk")
                    nc.sync.dma_start(
                        wk_sb[:], wk_ext[l].rearrange("(c p) m -> p c m", p=128))
                    wv_sb = ap.tile([128, KC, DQ], BF16, tag="wv")
                    nc.sync.dma_start(
                        wv_sb[:], wv_ext[l].rearrange("(c p) m -> p c m", p=128))
                    wo_sb = ap.tile([128, 2, D], BF16, tag="wo")
                    nc.sync.dma_start(
                        wo_sb[:], wo_ext[l].rearrange("(c p) m -> p c m", p=128))

                    hall_m = []
                    for b in range(B):
                        h_sb = load_h_sb(hp, hall[b], NCORES)
                        # ---- qkv for this batch (1024 tokens, 2 halves)
                        qTb = ap.tile([128, 2, S], BF16, tag="qTb")
                        kTb = ap.tile([128, 2, S], BF16, tag="kTb")
                        v_b = ap.tile([128, 8, DQ], BF16, tag="vb")
                        for rr in range(2):
                            cs = cosf[:, 512 * rr:512 * (rr + 1)]
                            sn = sinf[:, 512 * rr:512 * (rr + 1)]
                            for wsb, dstT in ((wq_sb, qTb), (wk_sb, kTb)):
                                for m in range(2):
                                    pq = psmm.tile([128, 512], F32, tag="mm")
                                    for kc in range(KC):
                                        nc.tensor.matmul(
                                            pq[:],
                                            wsb[:, kc, 128 * m:128 * (m + 1)],
                                            h_sb[:, kc,
                                                 512 * rr:512 * (rr + 1)],
                                            start=(kc == 0), stop=(kc == KC - 1))
                                    ta = ap2.tile([128, 512], F32, tag="ropea")
                                    nc.vector.tensor_mul(ta[:], pq[:], cs)
                                    tb = ap2.tile([128, 512], F32, tag="ropeb")
                                    nc.vector.tensor_mul(
                                        tb[0:64, :], pq[64:128, :], sn[0:64, :])
                                    nc.vector.tensor_mul(
                                        tb[64:128, :], pq[0:64, :],
                                        sn[64:128, :])
                                    nc.vector.tensor_add(
                                        dstT[:, m, 512 * rr:512 * (rr + 1)],
                                        ta[:], tb[:])
                            for t in range(4):
                                pv = psmm.tile([128, DQ], F32, tag="mm")
                                for kc in range(KC):
                                    nc.tensor.matmul(
                                        pv[:],
                                        h_sb[:, kc, 512 * rr + 128 * t:
                                             512 * rr + 128 * (t + 1)],
                                        wv_sb[:, kc, :],
                                        start=(kc == 0), stop=(kc == KC - 1))
                                nc.vector.tensor_copy(
                                    v_b[:, 4 * rr + t, :], pv[:])
                        # ---- attention for b, per local head
                        oT = ap.tile([128, 2, S], BF16, tag="oT")
                        for hh in range(2):
                            qh = qTb[:, hh, :]
                            kh = kTb[:, hh, :]
                            pT_sb = appt.tile([128, 8, S], BF16, tag="pT")
                            for i in range(8):
                                nv = 512 if i < 4 else 1024
                                sc = ap2.tile([128, S], F32, tag="sc", bufs=2)
                                for j in range(nv // 512):
                                    ps_s = psmm.tile([128, 512], F32, tag="mm")
                                    nc.tensor.matmul(
                                        ps_s[:], qh[:, 128 * i:128 * (i + 1)],
                                        kh[:, 512 * j:512 * (j + 1)],
                                        start=True, stop=True)
                                    if j == i // 4:
                                        nc.vector.tensor_add(
                                            sc[:, 512 * j:512 * (j + 1)],
                                            ps_s[:], cmask[i % 4][:])
                                    else:
                                        nc.vector.tensor_copy(
                                            sc[:, 512 * j:512 * (j + 1)],
                                            ps_s[:])
                                mx = small.tile([128, 1], F32, tag="mx")
                                nc.vector.reduce_max(
                                    mx[:], sc[:, :nv],
                                    axis=mybir.AxisListType.X)
                                nmx = small.tile([128, 1], F32, tag="nmx")
                                nc.vector.tensor_scalar_mul(
                                    nmx[:], mx[:], -SCALE)
                                pb = ap2.tile([128, S], BF16, tag="pb", bufs=2)
                                se = small.tile([128, 1], F32, tag="se")
                                nc.scalar.activation(
                                    pb[:, :nv], sc[:, :nv], Exp,
                                    bias=nmx[:], scale=SCALE, accum_out=se[:])
                                rse = small.tile([128, 1], F32, tag="rse")
                                nc.vector.reciprocal(rse[:], se[:])
                                nc.vector.tensor_scalar_mul(
                                    pb[:, :nv], pb[:, :nv], rse[:])
                                for tq in range(nv // 512):
                                    pt = pstr.tile([128, 4, 128], BF16,
                                                   tag="tr")
                                    for q in range(4):
                                        nc.tensor.transpose(
                                            pt[:, q, :],
                                            pb[:, 512 * tq + 128 * q:
                                               512 * tq + 128 * (q + 1)],
                                            ident[:])
                                    nc.vector.tensor_copy(
                                        pT_sb[:, 4 * tq:4 * (tq + 1),
                                              128 * i:128 * (i + 1)], pt[:])
                            for ns in range(2):
                                po = psmm.tile([128, 512], F32, tag="mm")
                                kmax = 4 if ns == 0 else 8
                                for tcb in range(kmax):
                                    nc.tensor.matmul(
                                        po[:],
                                        v_b[:, tcb, 128 * hh:128 * (hh + 1)],
                                        pT_sb[:, tcb, 512 * ns:512 * (ns + 1)],
                                        start=(tcb == 0), stop=(tcb == kmax - 1))
                                nc.vector.tensor_copy(
                                    oT[:, hh, 512 * ns:512 * (ns + 1)], po[:])

                        # ---- o_proj for batch b -> rs_in_b -> RS_b -> AG_b
                        rs_in_b = dram.tile([NCORES * 128, D], BF16,
                                            tag=f"rsi_a{l}_{b}",
                                            name=f"rsi_a{l}_{b}")
                        for c8 in range(NCORES):
                            for n in range(4):
                                pp = psmm.tile([128, 512], F32, tag="mm")
                                for kc in range(2):
                                    nc.tensor.matmul(
                                        pp[:],
                                        oT[:, kc, 128 * c8:128 * (c8 + 1)],
                                        wo_sb[:, kc, 512 * n:512 * (n + 1)],
                                        start=(kc == 0), stop=(kc == 1))
                                ob = drain.tile([128, 512], BF16, tag="ob")
                                if n % 2 == 0:
                                    nc.vector.tensor_copy(ob[:], pp[:])
                                else:
                                    nc.scalar.copy(ob[:], pp[:])
                                nc.sync.dma_start(
                                    rs_in_b[128 * c8:128 * (c8 + 1),
                                            512 * n:512 * (n + 1)], ob[:])
                        rs_out_b = reduce_scatter_q(rs_in_b, f"a{l}_{b}")
                        residual_add(b, rs_out_b)
                        hct_q = dram.tile([D, 128], BF16, tag=f"hct_m{l}_{b}",
                                          name=f"hct_m{l}_{b}")
                        norm_transpose(b, hct_q)
                        hall_m.append(allgather_q(hct_q, f"m{l}_{b}"))

                # ========================= MLP ==========================
                with tc.tile_pool(name=f"mlp{l}", bufs=1) as mp, \
                     tc.tile_pool(name=f"mlph{l}", bufs=1) as mhp, \
                     tc.tile_pool(name=f"mlp2{l}", bufs=2) as mp2, \
                     tc.tile_pool(name=f"mlpa{l}", bufs=1) as maT:
                    wg_sb = mp.tile([128, KC, FFC], BF16, tag="wg")
                    nc.sync.dma_start(
                        wg_sb[:], wg_ext[l].rearrange("(c p) m -> p c m", p=128))
                    wu_sb = mp.tile([128, KC, FFC], BF16, tag="wu")
                    nc.sync.dma_start(
                        wu_sb[:], wu_ext[l].rearrange("(c p) m -> p c m", p=128))

                    hall_n = []
                    for q in range(B):
                        h_sb = load_h_sb(mhp, hall_m[q], NCORES)
                        aT = maT.tile([128, 8, S], BF16, tag="aT")
                        for m in range(8):
                            for rr in range(2):
                                pg = psmm.tile([128, 512], F32, tag="mm")
                                for kc in range(KC):
                                    nc.tensor.matmul(
                                        pg[:],
                                        wg_sb[:, kc, 128 * m:128 * (m + 1)],
                                        h_sb[:, kc, 512 * rr:512 * (rr + 1)],
                                        start=(kc == 0), stop=(kc == KC - 1))
                                pu = psmm.tile([128, 512], F32, tag="mm")
                                for kc in range(KC):
                                    nc.tensor.matmul(
                                        pu[:],
                                        wu_sb[:, kc, 128 * m:128 * (m + 1)],
                                        h_sb[:, kc, 512 * rr:512 * (rr + 1)],
                                        start=(kc == 0), stop=(kc == KC - 1))
                                sg_t = mp2.tile([128, 512], F32, tag="silu")
                                nc.scalar.activation(sg_t[:], pg[:], Sigmoid)
                                nc.vector.tensor_mul(sg_t[:], sg_t[:], pg[:])
                                nc.vector.tensor_mul(
                                    aT[:, m, 512 * rr:512 * (rr + 1)],
                                    sg_t[:], pu[:])
                        rs_in_q = dram.tile([NCORES * 128, D], BF16,
                                            tag=f"rsi_m{l}_{q}",
                                            name=f"rsi_m{l}_{q}")
                        for n in range(4):
                            wdn = mp2.tile([128, 8, 512], BF16, tag="wdn")
                            nc.sync.dma_start(
                                wdn[:],
                                wd_ext[l, :, 512 * n:512 * (n + 1)].rearrange(
                                    "(c p) m -> p c m", p=128))
                            for t in range(8):
                                pd = psmm.tile([128, 512], F32, tag="mm")
                                for kc in range(8):
                                    nc.tensor.matmul(
                                        pd[:],
                                        aT[:, kc, 128 * t:128 * (t + 1)],
                                        wdn[:, kc, :],
                                        start=(kc == 0), stop=(kc == 7))
                                ob = drain.tile([128, 512], BF16, tag="ob")
                                if t % 2 == 0:
                                    nc.vector.tensor_copy(ob[:], pd[:])
                                else:
                                    nc.scalar.copy(ob[:], pd[:])
                                nc.sync.dma_start(
                                    rs_in_q[128 * t:128 * (t + 1),
                                            512 * n:512 * (n + 1)], ob[:])
                        rs_out_q = reduce_scatter_q(rs_in_q, f"m{l}_{q}")
                        residual_add(q, rs_out_q)
                        if l < L - 1:
                            hct_q = dram.tile([D, 128], BF16,
                                              tag=f"hct_a{l + 1}_{q}",
                                              name=f"hct_a{l + 1}_{q}")
                            norm_transpose(q, hct_q)
                            hall_n.append(allgather_q(hct_q, f"a{l + 1}_{q}"))
                    hall = hall_n

            # ===================== final norm + pool + head ==============
            with tc.tile_pool(name="fin", bufs=1) as finp:
                hw_sb = constp.tile([128, KC, NS], F32, tag="hw")
                nc.sync.dma_start(
                    hw_sb[:], hw_ext.ap().rearrange("(c p) m -> p c m", p=128))
                pvec = constp.tile([128, 128], BF16, tag="pvec")
                nc.vector.memset(pvec[:], 1.0 / S)
                pooled = finp.tile([1, 4 * D], F32, tag="pooled")
                for t in range(4):
                    hfin = finp.tile([128, D], BF16, tag=f"hfin{t}",
                                     name=f"hfin{t}")
                    rmsnorm_bf16(t, hfin)
                    for n in range(4):
                        pq = psq.tile([128, 512], F32, tag="pool",
                                      name=f"pool{t}_{n}")
                        nc.tensor.matmul(pq[:], pvec[:],
                                         hfin[:, 512 * n:512 * (n + 1)],
                                         start=True, stop=True)
                        nc.vector.tensor_copy(
                            pooled[0:1, D * t + 512 * n:D * t + 512 * (n + 1)],
                            pq[0:1, :])
                pdram = dram.tile([1, 4 * D], F32, tag="pdram", name="pdram")
                nc.sync.dma_start(pdram[:], pooled[:])
                pooledT = finp.tile([128, KC, 4], F32, tag="pooledT")
                for t in range(4):
                    nc.sync.dma_start(
                        pooledT[:, :, t],
                        pdram[0:1, D * t:D * (t + 1)].rearrange(
                            "a (c p) -> p (a c)", p=128))
                py = psq.tile([NS, 4], F32, tag="head")
                for kc in range(KC):
                    nc.tensor.matmul(
                        py[:], hw_sb[:, kc, :], pooledT[:, kc, :],
                        start=(kc == 0), stop=(kc == KC - 1))
                y_sb = finp.tile([NS, 4], F32, tag="y")
                nc.vector.tensor_copy(y_sb[:], py[:])
                nc.sync.dma_start(out_ext[:], y_sb[:])

    nc.compile()
    return nc


# ---------------------------------------------------------------- host prep
def _dequant(codes, scales):
    """codes uint8 [..., O, I], scales f32 [..., O, I//BLK] -> f32 [..., O, I]."""
    w = NF4[codes]
    shp = w.shape
    w = w.reshape(shp[:-1] + (shp[-1] // BLK, BLK)) * scales[..., None]
    return w.reshape(shp).astype(np.float32)


def prep_inputs(inputs):
    """Full inputs -> per-core in_maps."""
    ii = np.asarray(inputs["input_ids"])
    embed = np.asarray(inputs["embed"], dtype=np.float32)
    x0 = embed[ii].reshape(TOK, D)

    attn_w = _dequant(np.asarray(inputs["attn_codes"]),
                      np.asarray(inputs["attn_scales"], dtype=np.float32))
    gu_w = _dequant(np.asarray(inputs["gu_codes"]),
                    np.asarray(inputs["gu_scales"], dtype=np.float32))
    down_w = _dequant(np.asarray(inputs["down_codes"]),
                      np.asarray(inputs["down_scales"], dtype=np.float32))
    anw = np.asarray(inputs["attn_norm_w"], dtype=np.float32)   # [L, D]
    mnw = np.asarray(inputs["mlp_norm_w"], dtype=np.float32)    # [L, D]
    fnw = np.asarray(inputs["final_norm_w"], dtype=np.float32)  # [D]
    head_w = np.asarray(inputs["head_w"], dtype=np.float32)     # [NS, D]

    # rope basis permutation within each head: [even dims, odd dims]
    perm = np.concatenate([np.arange(0, DH, 2), np.arange(1, DH, 2)])

    # rope tables over in-batch positions 0..S-1
    inv = 1.0 / (ROPE_THETA ** (np.arange(0, DH, 2, dtype=np.float32) / DH))
    ang = inv[:, None] * np.arange(S, dtype=np.float32)[None, :]    # [64, S]
    cosf = np.concatenate([np.cos(ang), np.cos(ang)], axis=0).astype(BF)
    sinf = np.concatenate([-np.sin(ang), np.sin(ang)], axis=0).astype(BF)

    # causal mask tiles: cmask[d, sl, tl] = 0 if 128*d + sl >= tl else -1e9
    sl = np.arange(128)[:, None]
    tl = np.arange(512)[None, :]
    cmask = np.stack([np.where(128 * d + sl >= tl, 0.0, -1e9)
                      for d in range(4)]).astype(BF)

    hwp = (head_w * fnw[None, :]).T.copy().astype(np.float32)       # [D, NS]

    in_maps = []
    for c in range(NCORES):
        m = {}
        # strided token shard: batch-tile t = tokens [1024*t + 128*c, +128)
        xrows = np.concatenate([x0[1024 * t + 128 * c:1024 * t + 128 * (c + 1)]
                                for t in range(4)])
        m["x"] = np.ascontiguousarray(xrows)
        rows = slice(DQ * c, DQ * (c + 1))
        prows = np.concatenate([perm + DH * h for h in (0, 1)]) + DQ * c
        # fold input-side rmsnorm weight into the projection weights
        wq = np.stack([(attn_w[l, 0][prows] * anw[l][None, :]).T for l in range(L)])
        wk = np.stack([(attn_w[l, 1][prows] * anw[l][None, :]).T for l in range(L)])
        wv = np.stack([(attn_w[l, 2][rows] * anw[l][None, :]).T for l in range(L)])
        wo = np.stack([attn_w[l, 3][:, rows].T for l in range(L)])
        frows = slice(FFC * c, FFC * (c + 1))
        wg = np.stack([(gu_w[l, 0][frows] * mnw[l][None, :]).T for l in range(L)])
        wu = np.stack([(gu_w[l, 1][frows] * mnw[l][None, :]).T for l in range(L)])
        wd = np.stack([down_w[l][:, frows].T for l in range(L)])
        for k, wmat in (("wq", wq), ("wk", wk), ("wv", wv), ("wo", wo),
                        ("wg", wg), ("wu", wu), ("wd", wd)):
            m[k] = np.ascontiguousarray(wmat).astype(BF)
        m["cosf"] = cosf
        m["sinf"] = sinf
        m["cmask"] = cmask
        m["hw"] = hwp
        in_maps.append(m)
    return in_maps


def kernel(**inputs):
    from concourse.bass_utils import run_bass_kernel_spmd

    if "nc" not in _CACHE:
        _CACHE["nc"] = build_graph()
    nc = _CACHE["nc"]
    in_maps = prep_inputs(inputs)
    res = run_bass_kernel_spmd(nc, in_maps, core_ids=list(range(NCORES)))
    head_b = np.asarray(inputs["head_b"], dtype=np.float32)
    out = sum(res.results[c]["out"] for c in range(NCORES)).T  # [B, NS]
    return (out + head_b[None, :]).astype(np.float32)


# revision 12
# speedup vs baseline: 1.3515x; 1.0657x over previous
"""Self-contained TRN2 Bass kernel for nn_AESModel_42760694399363.

2-layer NF4-quantized transformer (B=4,S=1024,D=2048,FF=8192,H=16) + mean-pool
+ linear head.  Tensor-parallel across 8 NeuronCores: q/k/v/gate/up
column-sharded, o/down row-sharded.  The residual stream is sequence-sharded
STRIDED BY BATCH: core c owns tokens {1024*q + 128*c .. +128} for each batch
q.  This makes every boundary collective a per-batch 4MB ReduceScatter /
AllGather that pipelines behind the next batch's attention (or next batch's
MLP chunk), hiding nearly all collective latency.  Host does embedding gather
and NF4 dequant; all matmul/attention/norm FLOPs run on device in bf16 with
f32 accumulation.
"""
import sys

sys.path.insert(0, "/opt/trn_rl_repo")

import numpy as np
import ml_dtypes

# ---------------------------------------------------------------- constants
B, S, D, L, FF, V, NS = 4, 1024, 2048, 2, 8192, 32000, 11
H, DH = 16, 128
NCORES = 8
TOK = B * S              # 4096
SHARD = TOK // NCORES    # 512 tokens per core (4 tiles of 128, one per batch)
DQ = D // NCORES         # 256  q/k/v out-cols per core
FFC = FF // NCORES       # 1024 gate/up cols per core
KC = D // 128            # 16 contraction chunks over D
SCALE = 1.0 / np.sqrt(DH)
EPS = 1e-5
ROPE_THETA = 10000.0
BLK = 64

NF4 = np.array([-1.0, -0.6961928009986877, -0.5250730514526367, -0.39491748809814453,
                -0.28444138169288635, -0.18477343022823334, -0.09105003625154495, 0.0,
                0.07958029955625534, 0.16093020141124725, 0.24611230194568634,
                0.33791524171829224, 0.44070982933044434, 0.5626170039176941,
                0.7229568362236023, 1.0], dtype=np.float32)

BF = ml_dtypes.bfloat16

_CACHE = {}


# ---------------------------------------------------------------- device graph
def build_graph():
    import concourse.mybir as mybir
    import concourse.tile as tile
    from concourse import bacc
    from concourse.masks import make_identity
    from concourse.tile_rust import add_dep_helper

    F32 = mybir.dt.float32
    BF16 = mybir.dt.bfloat16
    RG = [list(range(NCORES))]
    Exp = mybir.ActivationFunctionType.Exp
    Sigmoid = mybir.ActivationFunctionType.Sigmoid

    nc = bacc.Bacc("TRN2", target_bir_lowering=False, debug=False,
                   num_devices=NCORES)
    _prev_cc = [None]

    def _chain_cc(cc):
        # never allow two collectives in flight: serialize in issue order
        if _prev_cc[0] is not None:
            add_dep_helper(cc.ins, _prev_cc[0], reason="serialize collectives")
        _prev_cc[0] = cc.ins

    x_ext = nc.declare_dram_parameter("x", [SHARD, D], F32, isOutput=False)
    wq_ext = nc.declare_dram_parameter("wq", [L, D, DQ], BF16, isOutput=False)
    wk_ext = nc.declare_dram_parameter("wk", [L, D, DQ], BF16, isOutput=False)
    wv_ext = nc.declare_dram_parameter("wv", [L, D, DQ], BF16, isOutput=False)
    wo_ext = nc.declare_dram_parameter("wo", [L, DQ, D], BF16, isOutput=False)
    wg_ext = nc.declare_dram_parameter("wg", [L, D, FFC], BF16, isOutput=False)
    wu_ext = nc.declare_dram_parameter("wu", [L, D, FFC], BF16, isOutput=False)
    wd_ext = nc.declare_dram_parameter("wd", [L, FFC, D], BF16, isOutput=False)
    cosf_ext = nc.declare_dram_parameter("cosf", [128, S], BF16, isOutput=False)
    sinf_ext = nc.declare_dram_parameter("sinf", [128, S], BF16, isOutput=False)
    cm_ext = nc.declare_dram_parameter("cmask", [4, 128, 512], BF16, isOutput=False)
    hw_ext = nc.declare_dram_parameter("hw", [D, NS], F32, isOutput=False)
    out_ext = nc.declare_dram_parameter("out", [NS, B], F32, isOutput=True)

    with tile.TileContext(nc) as tc:
        with tc.tile_pool(name="const", bufs=1) as constp, \
             tc.tile_pool(name="xres", bufs=1) as xres, \
             tc.tile_pool(name="norm", bufs=1) as normp, \
             tc.tile_pool(name="small", bufs=4) as small, \
             tc.tile_pool(name="drain", bufs=4) as drain, \
             tc.tile_pool(name="psmm", bufs=4, space="PSUM") as psmm, \
             tc.tile_pool(name="pstr", bufs=2, space="PSUM") as pstr, \
             tc.tile_pool(name="psq", bufs=1, space="PSUM") as psq, \
             tc.tile_pool(name="dram", bufs=1, space="DRAM") as dram:

            ident = constp.tile([128, 128], BF16, tag="ident")
            make_identity(nc, ident[:])

            # residual stream, f32, [128 part, q=batch tile, D]
            x_sb = xres.tile([128, 4, D], F32, tag="x")
            nc.sync.dma_start(
                x_sb[:], x_ext.ap().rearrange("(t p) d -> p t d", p=128))

            def rmsnorm_bf16(t, h_out):
                """h_out[128, D] bf16 = x_sb[:, t, :] * rsqrt(mean(x^2)+eps)."""
                ssq = small.tile([128, 1], F32, tag="ssq")
                nc.scalar.activation(h_out[:], x_sb[:, t, :],
                                     mybir.ActivationFunctionType.Square,
                                     accum_out=ssq[:])
                ms = small.tile([128, 1], F32, tag="ms")
                nc.vector.tensor_scalar(ms[:], ssq[:], 1.0 / D, EPS,
                                        mybir.AluOpType.mult,
                                        mybir.AluOpType.add)
                st = small.tile([128, 1], F32, tag="st")
                nc.scalar.sqrt(st[:], ms[:])
                rstd = small.tile([128, 1], F32, tag="rstd")
                nc.vector.reciprocal(rstd[:], st[:])
                nc.vector.tensor_scalar_mul(h_out[:], x_sb[:, t, :], rstd[:])

            def norm_transpose(t, hct_q):
                """rmsnorm batch-tile t -> 16 PE transposes -> hct_q [D, 128]."""
                hbf = normp.tile([128, D], BF16, tag="hbf")
                rmsnorm_bf16(t, hbf)
                for kq in range(KC // 4):
                    pt = pstr.tile([128, 4, 128], BF16, tag="tr")
                    for q in range(4):
                        nc.tensor.transpose(
                            pt[:, q, :],
                            hbf[:, 512 * kq + 128 * q:512 * kq + 128 * (q + 1)],
                            ident[:])
                    hts = drain.tile([128, 4, 128], BF16, tag="ob")
                    nc.vector.tensor_copy(hts[:], pt[:])
                    nc.sync.dma_start(
                        hct_q[512 * kq:512 * (kq + 1), :].rearrange(
                            "(q p) n -> p q n", p=128), hts[:])

            def allgather_q(hct_q, tagsuf):
                """AG [D,128] -> [NCORES*D, 128] (4MB)."""
                hall_q = dram.tile([NCORES * D, 128], BF16, tag="hall" + tagsuf,
                                   name="hall" + tagsuf, addr_space="Shared")
                cc = nc.gpsimd.collective_compute(
                    "AllGather", mybir.AluOpType.bypass, replica_groups=RG,
                    ins=[hct_q.opt()], outs=[hall_q.opt()])
                _chain_cc(cc)
                return hall_q

            def reduce_scatter_q(rs_in_q, tagsuf):
                """RS [8*128, D] -> [128, D] (in 4MB)."""
                rs_out_q = dram.tile([128, D], BF16, tag="rso" + tagsuf,
                                     name="rso" + tagsuf)
                cc = nc.gpsimd.collective_compute(
                    "ReduceScatter", mybir.AluOpType.add, replica_groups=RG,
                    ins=[rs_in_q.opt()], outs=[rs_out_q.opt()])
                _chain_cc(cc)
                return rs_out_q

            def residual_add(t, rs_out_q):
                """x_sb[:, t, :] += rs_out_q [128, D] (bf16 in DRAM)."""
                for n in range(4):
                    db = drain.tile([128, 512], BF16, tag="ob")
                    nc.sync.dma_start(
                        db[:], rs_out_q[:, 512 * n:512 * (n + 1)])
                    nc.vector.tensor_add(
                        x_sb[:, t, 512 * n:512 * (n + 1)],
                        x_sb[:, t, 512 * n:512 * (n + 1)], db[:])

            def load_h_sb(pool, hall_q, ncol):
                """Gathered h for one batch: [128, KC, ncol*128] from hall_q."""
                h_sb = pool.tile([128, KC, ncol * 128], BF16, tag="hr")
                for r in range(ncol):
                    nc.sync.dma_start(
                        h_sb[:, :, 128 * r:128 * (r + 1)],
                        hall_q[D * r:D * (r + 1), :].rearrange(
                            "(c p) n -> p c n", p=128))
                return h_sb

            pending = [None]

            def flush():
                if pending[0] is not None:
                    f = pending[0]
                    pending[0] = None
                    f()

            def make_post(b, rs_out_q, hct_name, agsuf, store):
                """Deferred post-RS work: residual add (+ norm/transpose/AG)."""
                def run():
                    residual_add(b, rs_out_q)
                    if store is not None:
                        hct_q = dram.tile([D, 128], BF16, tag=hct_name,
                                          name=hct_name)
                        norm_transpose(b, hct_q)
                        store[b] = allgather_q(hct_q, agsuf)
                return run

            # ---------------- initial: norm+transpose+AG each batch tile
            hall = {}
            for t in range(4):
                hct_q = dram.tile([D, 128], BF16, tag=f"hct_l0_{t}",
                                  name=f"hct_l0_{t}")
                norm_transpose(t, hct_q)
                hall[t] = allgather_q(hct_q, f"a0_{t}")

            for l in range(L):
                # ======================= attention =======================
                with tc.tile_pool(name=f"attn{l}", bufs=1) as ap, \
                     tc.tile_pool(name=f"attnh{l}", bufs=1) as hp, \
                     tc.tile_pool(name=f"attn2{l}", bufs=1) as ap2, \
                     tc.tile_pool(name=f"attnpT{l}", bufs=1) as appt:
                    cmask = [ap.tile([128, 512], BF16, tag=f"cmask{d}",
                                     name=f"cmask{l}_{d}") for d in range(4)]
                    for d in range(4):
                        nc.sync.dma_start(cmask[d][:], cm_ext[d])
                    cosf = ap.tile([128, S], BF16, tag="cosf")
                    nc.sync.dma_start(cosf[:], cosf_ext[:])
                    sinf = ap.tile([128, S], BF16, tag="sinf")
                    nc.sync.dma_start(sinf[:], sinf_ext[:])
                    wq_sb = ap.tile([128, KC, DQ], BF16, tag="wq")
                    nc.sync.dma_start(
                        wq_sb[:], wq_ext[l].rearrange("(c p) m -> p c m", p=128))
                    wk_sb = ap.tile([128, KC, DQ], BF16, tag="wk")
                    nc.sync.dma_start(
                        wk_sb[:], wk_ext[l].rearrange("(c p) m -> p c m", p=128))
                    wv_sb = ap.tile([128, KC, DQ], BF16, tag="wv")
                    nc.sync.dma_start(
                        wv_sb[:], wv_ext[l].rearrange("(c p) m -> p c m", p=128))
                    wo_sb = ap.tile([128, 2, D], BF16, tag="wo")
                    nc.sync.dma_start(
                        wo_sb[:], wo_ext[l].rearrange("(c p) m -> p c m", p=128))

                    hall_m = {}
                    for b in range(B):
                        h_sb = load_h_sb(hp, hall[b], NCORES)
                        # ---- qkv for this batch (1024 tokens, 2 halves)
                        qTb = ap.tile([128, 2, S], BF16, tag="qTb")
                        kTb = ap.tile([128, 2, S], BF16, tag="kTb")
                        v_b = ap.tile([128, 8, DQ], BF16, tag="vb")
                        for rr in range(2):
                            cs = cosf[:, 512 * rr:512 * (rr + 1)]
                            sn = sinf[:, 512 * rr:512 * (rr + 1)]
                            for wsb, dstT in ((wq_sb, qTb), (wk_sb, kTb)):
                                for m in range(2):
                                    pq = psmm.tile([128, 512], F32, tag="mm")
                                    for kc in range(KC):
                                        nc.tensor.matmul(
                                            pq[:],
                                            wsb[:, kc, 128 * m:128 * (m + 1)],
                                            h_sb[:, kc,
                                                 512 * rr:512 * (rr + 1)],
                                            start=(kc == 0), stop=(kc == KC - 1))
                                    ta = ap2.tile([128, 512], F32, tag="ropea")
                                    nc.vector.tensor_mul(ta[:], pq[:], cs)
                                    tb = ap2.tile([128, 512], F32, tag="ropeb")
                                    nc.vector.tensor_mul(
                                        tb[0:64, :], pq[64:128, :], sn[0:64, :])
                                    nc.vector.tensor_mul(
                                        tb[64:128, :], pq[0:64, :],
                                        sn[64:128, :])
                                    nc.vector.tensor_add(
                                        dstT[:, m, 512 * rr:512 * (rr + 1)],
                                        ta[:], tb[:])
                            for t in range(4):
                                pv = psmm.tile([128, DQ], F32, tag="mm")
                                for kc in range(KC):
                                    nc.tensor.matmul(
                                        pv[:],
                                        h_sb[:, kc, 512 * rr + 128 * t:
                                             512 * rr + 128 * (t + 1)],
                                        wv_sb[:, kc, :],
                                        start=(kc == 0), stop=(kc == KC - 1))
                                nc.vector.tensor_copy(
                                    v_b[:, 4 * rr + t, :], pv[:])
                        # ---- attention for b, per local head
                        oT = ap.tile([128, 2, S], BF16, tag="oT")
                        for hh in range(2):
                            if hh == 1:
                                flush()
                            qh = qTb[:, hh, :]
                            kh = kTb[:, hh, :]
                            pT_sb = appt.tile([128, 8, S], BF16, tag="pT")
                            for i in range(8):
                                nv = 512 if i < 4 else 1024
                                sc = ap2.tile([128, S], F32, tag="sc", bufs=2)
                                for j in range(nv // 512):
                                    ps_s = psmm.tile([128, 512], F32, tag="mm")
                                    nc.tensor.matmul(
                                        ps_s[:], qh[:, 128 * i:128 * (i + 1)],
                                        kh[:, 512 * j:512 * (j + 1)],
                                        start=True, stop=True)
                                    if j == i // 4:
                                        nc.vector.tensor_add(
                                            sc[:, 512 * j:512 * (j + 1)],
                                            ps_s[:], cmask[i % 4][:])
                                    else:
                                        nc.vector.tensor_copy(
                                            sc[:, 512 * j:512 * (j + 1)],
                                            ps_s[:])
                                mx = small.tile([128, 1], F32, tag="mx")
                                nc.vector.reduce_max(
                                    mx[:], sc[:, :nv],
                                    axis=mybir.AxisListType.X)
                                nmx = small.tile([128, 1], F32, tag="nmx")
                                nc.vector.tensor_scalar_mul(
                                    nmx[:], mx[:], -SCALE)
                                pb = ap2.tile([128, S], BF16, tag="pb", bufs=2)
                                se = small.tile([128, 1], F32, tag="se")
                                nc.scalar.activation(
                                    pb[:, :nv], sc[:, :nv], Exp,
                                    bias=nmx[:], scale=SCALE, accum_out=se[:])
                                rse = small.tile([128, 1], F32, tag="rse")
                                nc.vector.reciprocal(rse[:], se[:])
                                nc.vector.tensor_scalar_mul(
                                    pb[:, :nv], pb[:, :nv], rse[:])
                                for tq in range(nv // 512):
                                    pt = pstr.tile([128, 4, 128], BF16,
                                                   tag="tr")
                                    for q in range(4):
                                        nc.tensor.transpose(
                                            pt[:, q, :],
                                            pb[:, 512 * tq + 128 * q:
                                               512 * tq + 128 * (q + 1)],
                                            ident[:])
                                    nc.vector.tensor_copy(
                                        pT_sb[:, 4 * tq:4 * (tq + 1),
                                              128 * i:128 * (i + 1)], pt[:])
                            for ns in range(2):
                                po = psmm.tile([128, 512], F32, tag="mm")
                                kmax = 4 if ns == 0 else 8
                                for tcb in range(kmax):
                                    nc.tensor.matmul(
                                        po[:],
                                        v_b[:, tcb, 128 * hh:128 * (hh + 1)],
                                        pT_sb[:, tcb, 512 * ns:512 * (ns + 1)],
                                        start=(tcb == 0), stop=(tcb == kmax - 1))
                                nc.vector.tensor_copy(
                                    oT[:, hh, 512 * ns:512 * (ns + 1)], po[:])

                        # ---- o_proj for batch b -> rs_in_b -> RS_b -> AG_b
                        rs_in_b = dram.tile([NCORES * 128, D], BF16,
                                            tag=f"rsi_a{l}_{b}",
                                            name=f"rsi_a{l}_{b}")
                        for c8 in range(NCORES):
                            for n in range(4):
                                pp = psmm.tile([128, 512], F32, tag="mm")
                                for kc in range(2):
                                    nc.tensor.matmul(
                                        pp[:],
                                        oT[:, kc, 128 * c8:128 * (c8 + 1)],
                                        wo_sb[:, kc, 512 * n:512 * (n + 1)],
                                        start=(kc == 0), stop=(kc == 1))
                                ob = drain.tile([128, 512], BF16, tag="ob")
                                if n % 2 == 0:
                                    nc.vector.tensor_copy(ob[:], pp[:])
                                else:
                                    nc.scalar.copy(ob[:], pp[:])
                                nc.sync.dma_start(
                                    rs_in_b[128 * c8:128 * (c8 + 1),
                                            512 * n:512 * (n + 1)], ob[:])
                        rs_out_b = reduce_scatter_q(rs_in_b, f"a{l}_{b}")
                        pending[0] = make_post(b, rs_out_b, f"hct_m{l}_{b}",
                                               f"m{l}_{b}", hall_m)

                # ========================= MLP ==========================
                with tc.tile_pool(name=f"mlp{l}", bufs=1) as mp, \
                     tc.tile_pool(name=f"mlph{l}", bufs=1) as mhp, \
                     tc.tile_pool(name=f"mlp2{l}", bufs=2) as mp2, \
                     tc.tile_pool(name=f"mlpa{l}", bufs=1) as maT:
                    wg_sb = mp.tile([128, KC, FFC], BF16, tag="wg")
                    nc.sync.dma_start(
                        wg_sb[:], wg_ext[l].rearrange("(c p) m -> p c m", p=128))
                    wu_sb = mp.tile([128, KC, FFC], BF16, tag="wu")
                    nc.sync.dma_start(
                        wu_sb[:], wu_ext[l].rearrange("(c p) m -> p c m", p=128))

                    hall_n = {}
                    for q in range(B):
                        h_sb = load_h_sb(mhp, hall_m[q], NCORES)
                        aT = maT.tile([128, 8, S], BF16, tag="aT")
                        for m in range(8):
                            for rr in range(2):
                                pg = psmm.tile([128, 512], F32, tag="mm")
                                for kc in range(KC):
                                    nc.tensor.matmul(
                                        pg[:],
                                        wg_sb[:, kc, 128 * m:128 * (m + 1)],
                                        h_sb[:, kc, 512 * rr:512 * (rr + 1)],
                                        start=(kc == 0), stop=(kc == KC - 1))
                                pu = psmm.tile([128, 512], F32, tag="mm")
                                for kc in range(KC):
                                    nc.tensor.matmul(
                                        pu[:],
                                        wu_sb[:, kc, 128 * m:128 * (m + 1)],
                                        h_sb[:, kc, 512 * rr:512 * (rr + 1)],
                                        start=(kc == 0), stop=(kc == KC - 1))
                                sg_t = mp2.tile([128, 512], F32, tag="silu")
                                nc.scalar.activation(sg_t[:], pg[:], Sigmoid)
                                nc.vector.tensor_mul(sg_t[:], sg_t[:], pg[:])
                                nc.vector.tensor_mul(
                                    aT[:, m, 512 * rr:512 * (rr + 1)],
                                    sg_t[:], pu[:])
                        flush()
                        rs_in_q = dram.tile([NCORES * 128, D], BF16,
                                            tag=f"rsi_m{l}_{q}",
                                            name=f"rsi_m{l}_{q}")
                        for n in range(4):
                            wdn = mp2.tile([128, 8, 512], BF16, tag="wdn")
                            nc.sync.dma_start(
                                wdn[:],
                                wd_ext[l, :, 512 * n:512 * (n + 1)].rearrange(
                                    "(c p) m -> p c m", p=128))
                            for t in range(8):
                                pd = psmm.tile([128, 512], F32, tag="mm")
                                for kc in range(8):
                                    nc.tensor.matmul(
                                        pd[:],
                                        aT[:, kc, 128 * t:128 * (t + 1)],
                                        wdn[:, kc, :],
                                        start=(kc == 0), stop=(kc == 7))
                                ob = drain.tile([128, 512], BF16, tag="ob")
                                if t % 2 == 0:
                                    nc.vector.tensor_copy(ob[:], pd[:])
                                else:
                                    nc.scalar.copy(ob[:], pd[:])
                                nc.sync.dma_start(
                                    rs_in_q[128 * t:128 * (t + 1),
                                            512 * n:512 * (n + 1)], ob[:])
                        rs_out_q = reduce_scatter_q(rs_in_q, f"m{l}_{q}")
                        pending[0] = make_post(
                            q, rs_out_q, f"hct_a{l + 1}_{q}", f"a{l + 1}_{q}",
                            hall_n if l < L - 1 else None)
                    hall = hall_n

            # ===================== final norm + pool + head ==============
            with tc.tile_pool(name="fin", bufs=1) as finp:
                flush()
                hw_sb = constp.tile([128, KC, NS], F32, tag="hw")
                nc.sync.dma_start(
                    hw_sb[:], hw_ext.ap().rearrange("(c p) m -> p c m", p=128))
                pvec = constp.tile([128, 128], BF16, tag="pvec")
                nc.vector.memset(pvec[:], 1.0 / S)
                pooled = finp.tile([1, 4 * D], F32, tag="pooled")
                for t in range(4):
                    hfin = finp.tile([128, D], BF16, tag=f"hfin{t}",
                                     name=f"hfin{t}")
                    rmsnorm_bf16(t, hfin)
                    for n in range(4):
                        pq = psq.tile([128, 512], F32, tag="pool",
                                      name=f"pool{t}_{n}")
                        nc.tensor.matmul(pq[:], pvec[:],
                                         hfin[:, 512 * n:512 * (n + 1)],
                                         start=True, stop=True)
                        nc.vector.tensor_copy(
                            pooled[0:1, D * t + 512 * n:D * t + 512 * (n + 1)],
                            pq[0:1, :])
                pdram = dram.tile([1, 4 * D], F32, tag="pdram", name="pdram")
                nc.sync.dma_start(pdram[:], pooled[:])
                pooledT = finp.tile([128, KC, 4], F32, tag="pooledT")
                for t in range(4):
                    nc.sync.dma_start(
                        pooledT[:, :, t],
                        pdram[0:1, D * t:D * (t + 1)].rearrange(
                            "a (c p) -> p (a c)", p=128))
                py = psq.tile([NS, 4], F32, tag="head")
                for kc in range(KC):
                    nc.tensor.matmul(
                        py[:], hw_sb[:, kc, :], pooledT[:, kc, :],
                        start=(kc == 0), stop=(kc == KC - 1))
                y_sb = finp.tile([NS, 4], F32, tag="y")
                nc.vector.tensor_copy(y_sb[:], py[:])
                nc.sync.dma_start(out_ext[:], y_sb[:])

    nc.compile()
    return nc


# ---------------------------------------------------------------- host prep
def _dequant(codes, scales):
    """codes uint8 [..., O, I], scales f32 [..., O, I//BLK] -> f32 [..., O, I]."""
    w = NF4[codes]
    shp = w.shape
    w = w.reshape(shp[:-1] + (shp[-1] // BLK, BLK)) * scales[..., None]
    return w.reshape(shp).astype(np.float32)


def prep_inputs(inputs):
    """Full inputs -> per-core in_maps."""
    ii = np.asarray(inputs["input_ids"])
    embed = np.asarray(inputs["embed"], dtype=np.float32)
    x0 = embed[ii].reshape(TOK, D)

    attn_w = _dequant(np.asarray(inputs["attn_codes"]),
                      np.asarray(inputs["attn_scales"], dtype=np.float32))
    gu_w = _dequant(np.asarray(inputs["gu_codes"]),
                    np.asarray(inputs["gu_scales"], dtype=np.float32))
    down_w = _dequant(np.asarray(inputs["down_codes"]),
                      np.asarray(inputs["down_scales"], dtype=np.float32))
    anw = np.asarray(inputs["attn_norm_w"], dtype=np.float32)   # [L, D]
    mnw = np.asarray(inputs["mlp_norm_w"], dtype=np.float32)    # [L, D]
    fnw = np.asarray(inputs["final_norm_w"], dtype=np.float32)  # [D]
    head_w = np.asarray(inputs["head_w"], dtype=np.float32)     # [NS, D]

    # rope basis permutation within each head: [even dims, odd dims]
    perm = np.concatenate([np.arange(0, DH, 2), np.arange(1, DH, 2)])

    # rope tables over in-batch positions 0..S-1
    inv = 1.0 / (ROPE_THETA ** (np.arange(0, DH, 2, dtype=np.float32) / DH))
    ang = inv[:, None] * np.arange(S, dtype=np.float32)[None, :]    # [64, S]
    cosf = np.concatenate([np.cos(ang), np.cos(ang)], axis=0).astype(BF)
    sinf = np.concatenate([-np.sin(ang), np.sin(ang)], axis=0).astype(BF)

    # causal mask tiles: cmask[d, sl, tl] = 0 if 128*d + sl >= tl else -1e9
    sl = np.arange(128)[:, None]
    tl = np.arange(512)[None, :]
    cmask = np.stack([np.where(128 * d + sl >= tl, 0.0, -1e9)
                      for d in range(4)]).astype(BF)

    hwp = (head_w * fnw[None, :]).T.copy().astype(np.float32)       # [D, NS]

    in_maps = []
    for c in range(NCORES):
        m = {}
        # strided token shard: batch-tile t = tokens [1024*t + 128*c, +128)
        xrows = np.concatenate([x0[1024 * t + 128 * c:1024 * t + 128 * (c + 1)]
                                for t in range(4)])
        m["x"] = np.ascontiguousarray(xrows)
        rows = slice(DQ * c, DQ * (c + 1))
        prows = np.concatenate([perm + DH * h for h in (0, 1)]) + DQ * c
        # fold input-side rmsnorm weight into the projection weights
        wq = np.stack([(attn_w[l, 0][prows] * anw[l][None, :]).T for l in range(L)])
        wk = np.stack([(attn_w[l, 1][prows] * anw[l][None, :]).T for l in range(L)])
        wv = np.stack([(attn_w[l, 2][rows] * anw[l][None, :]).T for l in range(L)])
        wo = np.stack([attn_w[l, 3][:, rows].T for l in range(L)])
        frows = slice(FFC * c, FFC * (c + 1))
        wg = np.stack([(gu_w[l, 0][frows] * mnw[l][None, :]).T for l in range(L)])
        wu = np.stack([(gu_w[l, 1][frows] * mnw[l][None, :]).T for l in range(L)])
        wd = np.stack([down_w[l][:, frows].T for l in range(L)])
        for k, wmat in (("wq", wq), ("wk", wk), ("wv", wv), ("wo", wo),
                        ("wg", wg), ("wu", wu), ("wd", wd)):
            m[k] = np.ascontiguousarray(wmat).astype(BF)
        m["cosf"] = cosf
        m["sinf"] = sinf
        m["cmask"] = cmask
        m["hw"] = hwp
        in_maps.append(m)
    return in_maps


def kernel(**inputs):
    from concourse.bass_utils import run_bass_kernel_spmd

    if "nc" not in _CACHE:
        _CACHE["nc"] = build_graph()
    nc = _CACHE["nc"]
    in_maps = prep_inputs(inputs)
    res = run_bass_kernel_spmd(nc, in_maps, core_ids=list(range(NCORES)))
    head_b = np.asarray(inputs["head_b"], dtype=np.float32)
    out = sum(res.results[c]["out"] for c in range(NCORES)).T  # [B, NS]
    return (out + head_b[None, :]).astype(np.float32)
